# revision 13
# baseline (speedup 1.0000x reference)
"""Trainium2 Bass kernel for the neural-renderer silhouette MSE loss.

Reference computation: project 512 vertices, gather 1024 triangle faces,
rasterize a 256x256 silhouette (a pixel is covered iff it lies strictly
inside some valid face and the perspective-correct depth is in (NEAR, FAR)),
then return sum((sil - image_ref)^2).

Reformulation: each barycentric weight w_i of face f is an *affine* function
of the pixel NDC coords, w_i = a_i*x + b_i*y + c_i, so
    covered(p) = max_f min_i w_i(p, f) > 0.
The depth test is provably redundant when every camera-space vertex z lies
inside (NEAR, FAR); otherwise two extra affine maps are appended to the min.

Work pruning (host-side, exact):
  - A pixel strictly outside the global face bounding box can never be
    covered; its loss term ref^2 is summed on the host.
  - The in-bbox area is cut into 16x8-pixel blocks (= one 128-lane tile
    each). Each block only needs faces whose bbox overlaps it (~20 median,
    vs 1024). Blocks are sorted by face count and snake-dealt to the 8
    cores, so all cores run an identical slot schedule (SPMD) whose per-slot
    face capacity is the max count in the 8-block group.

Device (SPMD, one program on 8 cores; schedule baked at build time):
  - PE: per (slot, chunk): one K=9 bf16 matmul per affine map
        (lhsT = pixel matrix [9, 128], rhs = coefficients [9, ch]) -> PSUM.
    Each fp32 coefficient is split into 3 bf16 components (exact to ~2^-25);
    pixel coords (2i+1-256)/256 are exactly representable in bf16, so fp32
    PSUM accumulation reproduces fp32 affine values essentially exactly.
  - ACT: stages map 0 PSUM->SBUF as bf16 (sign-exact suffices) because the
    DVE reads at most one PSUM operand per instruction.
  - DVE: tensor_tensor mins + reduce_max over faces, then an epilogue
    computing sum((cov>0) - ref)^2 per partition row.
  - Host: sums 8x128 partials + the out-of-bbox ref^2 term.
"""

import os
import sys
from contextlib import ExitStack

import numpy as np

for _p in (
    "/opt/trn_rl_repo",
    "/root/.axon_site",
    "/root/.axon_site/_ro/trn_rl_repo",
    "/root/.axon_site/_ro/pypackages",
):
    if os.path.isdir(_p) and _p not in sys.path:
        sys.path.append(_p)

import ml_dtypes  # noqa: E402

import concourse.bacc as bacc  # noqa: E402
import concourse.bass as bass  # noqa: E402
import concourse.tile as tile  # noqa: E402
from concourse import mybir  # noqa: E402
from concourse.alu_op_type import AluOpType  # noqa: E402
from concourse.bass_utils import run_bass_kernel_spmd  # noqa: E402

IS = 256
NEAR, FAR = 0.1, 100.0
VIEW_ANGLE_DEG = 30.0
CAM_DIST, ELEV, AZIM = 2.732, 0.0, 90.0
EPS = 1e-9

NCORES = 8
PTILE = 128                  # pixels per tile slot (partition dim)
BH, BW = 16, 8               # pixel block shape (BH*BW == PTILE)
MAXCHUNK = 512               # max matmul free size / PSUM bank
KSPLIT = 3                   # bf16 components per fp32 coefficient
K = 3 * KSPLIT               # matmul contraction dim
DUMMY_XY = -4.0              # off-screen coord for padding pixels

_prog_cache: dict = {}


def _camera_transform(v: np.ndarray) -> np.ndarray:
    """Replicate reference's look_at + perspective in fp32. v: [V,3]."""
    e, a = np.radians(ELEV), np.radians(AZIM)
    eye = np.array(
        [
            CAM_DIST * np.cos(e) * np.sin(a),
            CAM_DIST * np.sin(e),
            -CAM_DIST * np.cos(e) * np.cos(a),
        ],
        dtype=np.float32,
    )
    at = np.zeros(3, np.float32)
    up = np.array([0.0, 1.0, 0.0], np.float32)
    z = at - eye
    z = (z / np.linalg.norm(z)).astype(np.float32)
    x = np.cross(up, z)
    x = (x / np.linalg.norm(x)).astype(np.float32)
    y = np.cross(z, x)
    y = (y / np.linalg.norm(y)).astype(np.float32)
    R = np.stack([x, y, z]).astype(np.float32)
    vc = ((v - eye) @ R.T).astype(np.float32)
    w = np.float32(np.tan(np.radians(VIEW_ANGLE_DEG)))
    zc = vc[:, 2]
    return np.stack([vc[:, 0] / (zc * w), vc[:, 1] / (zc * w), zc], -1).astype(
        np.float32
    )


def _face_coefficients(fv: np.ndarray):
    """Affine coefficients per map: returns (coeffs [nmaps,3,F] f32,
    valid [F] bool, nmaps)."""
    F = fv.shape[0]
    x0, x1, x2 = fv[:, 0, 0], fv[:, 1, 0], fv[:, 2, 0]
    y0, y1, y2 = fv[:, 0, 1], fv[:, 1, 1], fv[:, 2, 1]
    z0, z1, z2 = fv[:, 0, 2], fv[:, 1, 2], fv[:, 2, 2]

    denom = (y1 - y2) * (x0 - x2) + (x2 - x1) * (y0 - y2)
    valid = (np.abs(denom) > EPS) & np.all(np.isfinite(fv.reshape(F, -1)), -1)
    d = np.where(valid, denom, np.float32(1.0)).astype(np.float32)

    a0 = (y1 - y2) / d
    b0 = (x2 - x1) / d
    c0 = -(a0 * x2 + b0 * y2)
    a1 = (y2 - y0) / d
    b1 = (x0 - x2) / d
    c1 = -(a1 * x2 + b1 * y2)
    a2 = -(a0 + a1)
    b2 = -(b0 + b1)
    c2 = np.float32(1.0) - c0 - c1

    # Depth redundancy: for an interior pixel the perspective-correct depth
    # is a harmonic mean of vertex z's, hence inside (NEAR, FAR) whenever
    # all (valid-face) vertex z's are.
    z_valid = fv[valid][:, :, 2] if valid.any() else np.array([[1.0]])
    depth_safe = bool(
        np.all((z_valid > NEAR * 1.0001) & (z_valid < FAR * 0.9999)))

    maps = [(a0, b0, c0), (a1, b1, c1), (a2, b2, c2)]
    if not depth_safe:
        iz0 = np.float32(1.0) / z0
        iz1 = np.float32(1.0) / z1
        iz2 = np.float32(1.0) / z2
        az = a0 * iz0 + a1 * iz1 + a2 * iz2
        bz = b0 * iz0 + b1 * iz1 + b2 * iz2
        cz = c0 * iz0 + c1 * iz1 + c2 * iz2
        maps.append((az, bz, cz - np.float32(1.0 / FAR)))
        maps.append((-az, -bz, np.float32(1.0 / NEAR) - cz))

    nmaps = len(maps)
    coeffs = np.empty((nmaps, 3, F), np.float32)
    for m, (a, b, c) in enumerate(maps):
        bad = ~(valid & np.isfinite(a) & np.isfinite(b) & np.isfinite(c))
        coeffs[m, 0] = np.where(bad, np.float32(0.0), a)
        coeffs[m, 1] = np.where(bad, np.float32(0.0), b)
        coeffs[m, 2] = np.where(bad, np.float32(-1.0), c)
    return coeffs, valid, nmaps


def _split_bf16(v: np.ndarray) -> list[np.ndarray]:
    """Split fp32 array into KSPLIT bf16 components summing to ~v (2^-25)."""
    parts = []
    rem = v.astype(np.float32)
    for _ in range(KSPLIT):
        p = rem.astype(ml_dtypes.bfloat16)
        parts.append(p)
        rem = (rem - p.astype(np.float32)).astype(np.float32)
    return parts


def _make_schedule(vertices, image_ref, faces):
    """Host planning: prune + block + deal. Returns (in_maps, nmaps,
    chunks_per_slot, host_extra)."""
    v = np.asarray(vertices, np.float32)[0]
    f = np.asarray(faces)[0].astype(np.int64)
    img = np.asarray(image_ref, np.float32)[0]
    img_flat = img.reshape(-1)

    vp = _camera_transform(v)
    fv = vp[f]                                    # [F,3,3]
    coeffs, valid, nmaps = _face_coefficients(fv)
    F = fv.shape[0]

    i = np.arange(IS, dtype=np.float32)
    xcol = (2.0 * i + 1.0 - IS) / IS
    yrow = (2.0 * (IS - 1.0 - i) + 1.0 - IS) / IS   # decreasing in row
    marg = np.float32(2.0 / IS)                     # one-pixel margin

    vi = np.where(valid)[0]
    if len(vi):
        fx = fv[:, :, 0]
        fy = fv[:, :, 1]
        fxmin, fxmax = fx.min(1), fx.max(1)
        fymin, fymax = fy.min(1), fy.max(1)
        gxmin, gxmax = fxmin[vi].min(), fxmax[vi].max()
        gymin, gymax = fymin[vi].min(), fymax[vi].max()
        rows = np.where((yrow >= gymin - marg) & (yrow <= gymax + marg))[0]
        cols = np.where((xcol >= gxmin - marg) & (xcol <= gxmax + marg))[0]
    else:
        rows = cols = np.array([], np.int64)

    blocks = []   # (count, face_idx_array, pixel_idx_array (len<=128))
    if len(rows) and len(cols):
        r0, r1 = int(rows.min()), int(rows.max()) + 1
        c0, c1 = int(cols.min()), int(cols.max()) + 1
        for rr in range(r0, r1, BH):
            for cc in range(c0, c1, BW):
                rr2, cc2 = min(rr + BH, r1), min(cc + BW, c1)
                ylo, yhi = yrow[rr2 - 1] - marg, yrow[rr] + marg
                xlo, xhi = xcol[cc] - marg, xcol[cc2 - 1] + marg
                inter = valid & (fymax >= ylo) & (fymin <= yhi) \
                    & (fxmax >= xlo) & (fxmin <= xhi)
                fl = np.where(inter)[0]
                rgrid, cgrid = np.meshgrid(np.arange(rr, rr2),
                                           np.arange(cc, cc2), indexing="ij")
                px = (rgrid * IS + cgrid).reshape(-1)
                blocks.append((len(fl), fl, px))

    if not blocks:
        blocks = [(0, np.array([], np.int64), np.array([], np.int64))]

    blocks.sort(key=lambda b: -b[0])
    NT = (len(blocks) + NCORES - 1) // NCORES
    empty = (0, np.array([], np.int64), np.array([], np.int64))
    while len(blocks) < NT * NCORES:
        blocks.append(empty)

    # slot capacities and chunk splits (shared across cores)
    chunks_per_slot = []
    for j in range(NT):
        grp = blocks[NCORES * j:NCORES * (j + 1)]
        cap = max(32, int(np.ceil(max(b[0] for b in grp) / 32)) * 32)
        nch = (cap + MAXCHUNK - 1) // MAXCHUNK
        ch = int(np.ceil(cap / nch / 32)) * 32
        chunks_per_slot.append((ch,) * nch)
    chunks_per_slot = tuple(chunks_per_slot)
    CTOT = sum(sum(c) for c in chunks_per_slot)

    # coefficient splits with a trailing dummy column (index F)
    csp = np.empty((nmaps, 3, KSPLIT, F + 1), ml_dtypes.bfloat16)
    for m in range(nmaps):
        for j3 in range(3):
            col = np.concatenate(
                [coeffs[m, j3],
                 [np.float32(-1.0 if j3 == 2 else 0.0)]])
            for s, part in enumerate(_split_bf16(col)):
                csp[m, j3, s] = part

    assigned = np.zeros(IS * IS, bool)
    in_maps = []
    for k in range(NCORES):
        pix = np.full((K, NT * PTILE), np.float32(DUMMY_XY), np.float32)
        ref = np.zeros((PTILE, NT), np.float32)
        coef = np.empty((K, nmaps * CTOT), ml_dtypes.bfloat16)
        colbase = 0
        for j in range(NT):
            cnt, fl, px = blocks[NCORES * j + k]
            # pixels
            npx = len(px)
            if npx:
                lane_x = xcol[px % IS]
                lane_y = yrow[px // IS]
                for s in range(KSPLIT):
                    pix[s * 3 + 0, j * PTILE:j * PTILE + npx] = lane_x
                    pix[s * 3 + 1, j * PTILE:j * PTILE + npx] = lane_y
                ref[:npx, j] = img_flat[px]
                assigned[px] = True
            for s in range(KSPLIT):
                pix[s * 3 + 2, j * PTILE:(j + 1) * PTILE] = 1.0
            # faces (padded with dummy index F)
            capj = sum(chunks_per_slot[j])
            fidx = np.full(capj, F, np.int64)
            fidx[:cnt] = fl
            pos = 0
            for ch in chunks_per_slot[j]:
                sel = fidx[pos:pos + ch]
                for m in range(nmaps):
                    for s in range(KSPLIT):
                        for j3 in range(3):
                            coef[s * 3 + j3,
                                 colbase + m * ch:colbase + (m + 1) * ch] = \
                                csp[m, j3, s][sel]
                colbase += nmaps * ch
                pos += ch
        in_maps.append({
            "coef": coef,
            "pix": pix.astype(ml_dtypes.bfloat16),
            "ref": ref,
        })

    host_extra = float(np.sum(np.square(img_flat[~assigned]),
                              dtype=np.float32))
    return in_maps, nmaps, chunks_per_slot, host_extra


def _build_program(nmaps: int, chunks_per_slot) -> bass.Bass:
    NT = len(chunks_per_slot)
    CTOT = sum(sum(c) for c in chunks_per_slot)
    nc = bacc.Bacc()
    coef_d = nc.dram_tensor("coef", [K, nmaps * CTOT], mybir.dt.bfloat16,
                            kind="ExternalInput")
    pix_d = nc.dram_tensor("pix", [K, NT * PTILE], mybir.dt.bfloat16,
                           kind="ExternalInput")
    ref_d = nc.dram_tensor("ref", [PTILE, NT], mybir.dt.float32,
                           kind="ExternalInput")
    out_d = nc.dram_tensor("out", [PTILE, 1], mybir.dt.float32,
                           kind="ExternalOutput")

    # slot -> coef column span (in the nmaps*CTOT axis)
    slot_cols = []
    cb = 0
    for j in range(NT):
        w = nmaps * sum(chunks_per_slot[j])
        slot_cols.append((cb, cb + w))
        cb += w

    # group slots into DMA parts of roughly equal column width so transfers
    # run on parallel queues and early slots can start before later parts land
    NPART = min(4, NT)
    target = cb / NPART
    part_of_slot = [min(NPART - 1, int(slot_cols[j][0] // target))
                    for j in range(NT)]
    part_ranges = []
    for g in range(NPART):
        sl = [j for j in range(NT) if part_of_slot[j] == g]
        if sl:
            part_ranges.append((slot_cols[sl[0]][0], slot_cols[sl[-1]][1]))
        else:
            part_ranges.append(None)

    with tile.TileContext(nc) as tc:
        with ExitStack() as ctx:
            const = ctx.enter_context(tc.tile_pool(name="const", bufs=1))
            # pix first (every slot reads it), then coef parts; one issue per
            # engine queue so the ~700ns per-issue cost runs in parallel
            pix_s = const.tile([K, NT * PTILE], mybir.dt.bfloat16)
            nc.sync.dma_start(pix_s[:], pix_d[:])
            issue_engines = [nc.gpsimd, nc.scalar, nc.sync]
            coef_parts = []
            for g, rng in enumerate(part_ranges):
                if rng is None:
                    coef_parts.append(None)
                    continue
                lo, hi = rng
                cp = const.tile([K, hi - lo], mybir.dt.bfloat16,
                                name=f"coefp{g}")
                issue_engines[g % len(issue_engines)].dma_start(
                    cp[:], coef_d[:, lo:hi])
                coef_parts.append((cp, lo))
            ref_s = const.tile([PTILE, NT], mybir.dt.float32)
            nc.gpsimd.dma_start(ref_s[:], ref_d[:])
            mx = const.tile([PTILE, NT], mybir.dt.float32)
            nextra = sum(len(c) - 1 for c in chunks_per_slot)
            extra = const.tile([PTILE, max(nextra, 1)], mybir.dt.float32)

            psum = ctx.enter_context(
                tc.tile_pool(name="psum", bufs=2, space="PSUM"))
            tmp = ctx.enter_context(tc.tile_pool(name="tmp", bufs=3))

            colbase = 0
            eidx = 0
            for j in range(NT):
                lhsT = pix_s[:, j * PTILE:(j + 1) * PTILE]
                cpart, cplo = coef_parts[part_of_slot[j]]
                for ci, ch in enumerate(chunks_per_slot[j]):
                    ws = []
                    for m in range(nmaps):
                        w = psum.tile([PTILE, MAXCHUNK], mybir.dt.float32,
                                      tag=f"w{m}", bufs=(2 if m < 3 else 1))
                        lo = colbase - cplo + m * ch
                        rhs = cpart[:, lo:lo + ch]
                        nc.tensor.matmul(w[:, :ch], lhsT, rhs,
                                         start=True, stop=True)
                        ws.append(w)
                    colbase += nmaps * ch
                    # ACT stages map0 (DVE: single PSUM operand per inst)
                    w0c = tmp.tile([PTILE, MAXCHUNK], mybir.dt.bfloat16,
                                   tag="w0c")
                    nc.scalar.copy(w0c[:, :ch], ws[0][:, :ch])
                    mn = tmp.tile([PTILE, MAXCHUNK], mybir.dt.bfloat16,
                                  tag="mn")
                    nc.vector.tensor_tensor(mn[:, :ch], w0c[:, :ch],
                                            ws[1][:, :ch], op=AluOpType.min)
                    for m in range(2, nmaps):
                        nc.vector.tensor_tensor(mn[:, :ch], mn[:, :ch],
                                                ws[m][:, :ch],
                                                op=AluOpType.min)
                    if ci == 0:
                        dst = mx[:, j:j + 1]
                    else:
                        dst = extra[:, eidx:eidx + 1]
                    nc.vector.reduce_max(dst, mn[:, :ch],
                                         axis=mybir.AxisListType.X)
                    if ci > 0:
                        nc.vector.tensor_tensor(mx[:, j:j + 1], mx[:, j:j + 1],
                                                extra[:, eidx:eidx + 1],
                                                op=AluOpType.max)
                        eidx += 1

            # diff = (mx > 0 ? 1.0 : 0.0) - ref ; out = rowsum(diff^2)
            diff = const.tile([PTILE, NT], mybir.dt.float32)
            nc.vector.scalar_tensor_tensor(
                out=diff[:], in0=mx[:], scalar=0.0, in1=ref_s[:],
                op0=AluOpType.is_gt, op1=AluOpType.subtract)
            sq = const.tile([PTILE, NT], mybir.dt.float32)
            nc.vector.tensor_tensor(sq[:], diff[:], diff[:],
                                    op=AluOpType.mult)
            losscol = const.tile([PTILE, 1], mybir.dt.float32)
            nc.vector.reduce_sum(losscol[:], sq[:],
                                 axis=mybir.AxisListType.X)
            nc.sync.dma_start(out_d[:], losscol[:])
    nc.compile()
    return nc


def run_sharded(vertices, image_ref, faces, trace=False, **spmd_kwargs):
    """Runs the SPMD kernel on 8 cores; returns (loss, BassKernelResults)."""
    in_maps, nmaps, chunks, host_extra = _make_schedule(
        vertices, image_ref, faces)
    key = (nmaps, chunks)
    if key not in _prog_cache:
        _prog_cache[key] = _build_program(nmaps, chunks)
    nc = _prog_cache[key]
    results = run_bass_kernel_spmd(
        nc, in_maps, core_ids=list(range(NCORES)), trace=trace, **spmd_kwargs)
    partials = np.stack([r["out"].reshape(-1) for r in results.results])
    loss = np.float32(partials.astype(np.float32).sum(dtype=np.float32)
                      + np.float32(host_extra))
    return loss, results


def kernel(vertices: np.ndarray, image_ref: np.ndarray,
           faces: np.ndarray) -> np.ndarray:
    loss, _ = run_sharded(vertices, image_ref, faces, trace=False)
    return np.asarray(loss, dtype=np.float32)
